# revision 2
# baseline (speedup 1.0000x reference)
"""AttentionPooling (segment softmax-pool) Trainium2 Bass kernel. v2

out[g, :] = sum_{i: batch[i]==g} softmax_within_segment(score)_i * x[i, :]
score_i = tanh(x_i @ W1 + b1) @ W2 + b2

Math notes:
- softmax is shift-invariant, so b2 and the per-segment max subtraction
  cancel exactly; we compute e_i = exp(s_i) with s_i = tanh(xW1+b1)@W2
  and normalize by the per-segment sum of e at the very end.
- per-segment sums run on the TensorEngine: for each 128-node subtile,
  a one-hot matrix ow[i, g] = e_i * (batch_local[i] == g) is built with
  one DVE tensor_scalar op, then ow.T @ [x | 1] accumulates
  [num_local_segments, D+1] into ONE persistent PSUM accumulation group
  spanning the whole kernel (column D is the softmax denominator).

Precision: pooling consumes x in bf16; the score MLP consumes a second,
transposed copy of x in fp8 e3m4 (softmax weights tolerate ~2% score
noise; validated rel_err ~8e-3 vs the f32 reference). The score matmul
runs mixed-dtype: bf16 W1 stationary x fp8e3 moving.

HBM traffic per core: 12.9 MB (natural bf16 + ones col) + 6.3 MB
(transposed fp8) = 19.2 MB, moved with ~0.5-2 MB grouped DMAs (each
DMA instruction is split across all 16 SDMA engines).

Sharding: nodes split across 8 cores at segment boundaries (batch is
sorted); each core reduces its own segments; host concatenates the
per-core [G_c, D] outputs.
"""

import sys

sys.path.insert(0, "/opt/trn_rl_repo")

import numpy as np
import ml_dtypes

import concourse.bass as bass
import concourse.tile as tile
from concourse import mybir
from concourse.bass_utils import run_bass_kernel_spmd

BF16 = ml_dtypes.bfloat16
E3M4 = ml_dtypes.float8_e3m4

N_CORES = 8
D = 256
H = 128  # hidden dim of the score MLP
C = 512  # nodes per chunk
SUB = C // 128


def _split_multiwait(nc):
    """Split multi-wait instructions for this walrus build.

    This neuronxcc/walrus rejects more than one sync-wait command per
    instruction ("Too many sync wait commands"), but tile emits 2-3 waits
    on compute/DMA instructions and many on the final Drain. Hoist the
    extra waits onto preceding InstEventSemaphore instructions (the native
    sequencer wait primitive, 2 waits each) on the same engine. Engine
    program order makes this equivalent: the stream blocks on the EVSEM
    waits, then on the instruction's remaining wait.
    """
    for bb in nc.main_func.blocks:
        new = []
        for ins in bb.instructions:
            w = (
                list(ins.sync_info.on_wait)
                if (ins.sync_info and ins.sync_info.on_wait)
                else []
            )
            if len(w) > 1:
                extras = w[:-1]
                for i in range(0, len(extras), 2):
                    ev = mybir.InstEventSemaphore(
                        name=nc.get_next_instruction_name(),
                        engine=ins.engine,
                        sync_info=mybir.SyncInfo(
                            on_wait=extras[i : i + 2], on_update=[]
                        ),
                    )
                    nc.register_instruction(ev)
                    new.append(ev)
                ins.sync_info.on_wait = [w[-1]]
            new.append(ins)
        bb.instructions[:] = new


def _group_plan(T):
    """DMA grouping: a small first group to prime the pipeline, then big."""
    plan = []
    t = 0
    first = min(2, T)
    plan.append((0, first))
    t = first
    while t < T:
        n = min(8, T - t)
        plan.append((t, t + n))
        t += n
    return plan


def _build_program(T, GM):
    """Build the SPMD Bass program: T chunks of C nodes, GM local segments."""
    f32 = mybir.dt.float32
    bf16 = mybir.dt.bfloat16
    fp8 = mybir.dt.float8e3
    T4 = T * SUB

    nc = bass.Bass(trn_type="TRN2")
    xn = nc.dram_tensor("xn", [128, T, SUB, D + 1], bf16, kind="ExternalInput")
    xt = nc.dram_tensor("xt", [128, T, 2, C], fp8, kind="ExternalInput")
    w1 = nc.dram_tensor("w1", [2, 128, H], bf16, kind="ExternalInput")
    w2 = nc.dram_tensor("w2", [H, 1], bf16, kind="ExternalInput")
    b1v = nc.dram_tensor("b1v", [H, 1], f32, kind="ExternalInput")
    blo = nc.dram_tensor("blo", [128, T4], f32, kind="ExternalInput")
    iot = nc.dram_tensor("iot", [128, GM], f32, kind="ExternalInput")
    dbi = nc.dram_tensor("dbi", [GM, 1], f32, kind="ExternalInput")
    out = nc.dram_tensor("out", [GM, D], f32, kind="ExternalOutput")

    Exp = mybir.ActivationFunctionType.Exp
    Tanh = mybir.ActivationFunctionType.Tanh

    plan = _group_plan(T)

    with tile.TileContext(nc) as tc:
        with (
            tc.tile_pool(name="const", bufs=1) as const,
            tc.tile_pool(name="work", bufs=8) as work,
            tc.tile_pool(name="hps", bufs=3, space="PSUM") as hps,
            tc.tile_pool(name="sps", bufs=2, space="PSUM") as sps,
            tc.tile_pool(name="accp", bufs=1, space="PSUM") as accp,
        ):
            w1sb = const.tile([128, 2, H], bf16)
            nc.sync.dma_start(out=w1sb[:, 0, :], in_=w1[0])
            nc.sync.dma_start(out=w1sb[:, 1, :], in_=w1[1])
            w2sb = const.tile([H, 1], bf16)
            nc.sync.dma_start(out=w2sb, in_=w2[:, :])
            b1sb = const.tile([H, 1], f32)
            nc.sync.dma_start(out=b1sb, in_=b1v[:, :])
            blsb = const.tile([128, T4], f32)
            nc.sync.dma_start(out=blsb, in_=blo[:, :])
            iosb = const.tile([128, GM], f32)
            nc.sync.dma_start(out=iosb, in_=iot[:, :])
            dbsb = const.tile([GM, 1], f32)
            nc.sync.dma_start(out=dbsb, in_=dbi[:, :])

            # grouped bulk loads of x (few big DMAs; each split over the
            # 16 SDMA engines). Separate tiles per group so chunk j only
            # depends on its own group's DMA.
            xtg, xng, gidx = [], [], []
            for gi, (t0, t1) in enumerate(plan):
                gt = t1 - t0
                xtt = const.tile([128, gt, 2, C], fp8)
                nc.sync.dma_start(out=xtt, in_=xt[:, t0:t1])
                xnt = const.tile([128, gt, SUB, D + 1], bf16)
                nc.sync.dma_start(out=xnt, in_=xn[:, t0:t1])
                xtg.append(xtt)
                xng.append(xnt)
                for lt in range(gt):
                    gidx.append((gi, lt))

            # persistent PSUM accumulator: one accumulation group spans
            # every pooling matmul in the kernel.
            pch = accp.tile([GM, D + 1], f32)

            for j in range(T):
                gi, lt = gidx[j]
                # hT[hidden, node] = sum_d W1[d, hidden] * x[node, d]
                hp = hps.tile([H, C], f32)
                nc.tensor.matmul(
                    hp, lhsT=w1sb[:, 0, :], rhs=xtg[gi][:, lt, 0, :],
                    start=True, stop=False,
                )
                nc.tensor.matmul(
                    hp, lhsT=w1sb[:, 1, :], rhs=xtg[gi][:, lt, 1, :],
                    start=False, stop=True,
                )
                tht = work.tile([H, C], bf16)
                nc.scalar.activation(tht, hp, Tanh, bias=b1sb[:, 0:1])

                # s[node] = sum_h tanh_h[h, node] * W2[h]; each subtile's
                # scores land in their own PSUM column. memset + start=False
                # keeps columns independent of stale has_written bits; the
                # final stop=True closes the group so Exp has a real
                # dependency edge (the baseline raced here).
                sp = sps.tile([128, SUB], f32)
                nc.vector.memset(sp, 0.0)
                for a in range(SUB):
                    nc.tensor.matmul(
                        sp[:, a : a + 1],
                        lhsT=tht[:, a * 128 : (a + 1) * 128],
                        rhs=w2sb,
                        start=False,
                        stop=(a == SUB - 1),
                        skip_group_check=True,
                    )
                et = work.tile([128, SUB], f32)
                nc.scalar.activation(et, sp, Exp)

                owt = work.tile([128, SUB, GM], bf16)
                for a in range(SUB):
                    # ow[i, g] = (iota[g] == batch_local[i]) * e[i]
                    t4 = j * SUB + a
                    nc.vector.tensor_scalar(
                        out=owt[:, a, :],
                        in0=iosb,
                        scalar1=blsb[:, t4 : t4 + 1],
                        scalar2=et[:, a : a + 1],
                        op0=mybir.AluOpType.is_equal,
                        op1=mybir.AluOpType.mult,
                    )
                    nc.tensor.matmul(
                        pch,
                        lhsT=owt[:, a, :],
                        rhs=xng[gi][:, lt, a, :],
                        start=(j == 0 and a == 0),
                        stop=(j == T - 1 and a == SUB - 1),
                        skip_group_check=True,
                    )

            # normalize: out[g, :] = pooled[g, :D] / (denom[g] + empty_guard)
            dn = work.tile([GM, 1], f32)
            nc.vector.tensor_scalar_add(dn, pch[:, D : D + 1], dbsb[:, 0:1])
            rc = work.tile([GM, 1], f32)
            nc.vector.reciprocal(rc, dn)
            ot = work.tile([GM, D], f32)
            nc.vector.tensor_scalar_mul(ot, pch[:, 0:D], rc[:, 0:1])
            nc.sync.dma_start(out=out[:, :], in_=ot)

    _split_multiwait(nc)
    return nc


def _prepare(inputs):
    """Host-side sharding and input staging. Returns (meta, in_maps)."""
    x = np.asarray(inputs["x"], dtype=np.float32)
    batch = np.asarray(inputs["batch"]).astype(np.int64)
    W1 = np.asarray(inputs["W1"], dtype=np.float32)
    b1 = np.asarray(inputs["b1"], dtype=np.float32)
    W2 = np.asarray(inputs["W2"], dtype=np.float32)

    n, d = x.shape
    assert d == D
    G = 512
    seg_ptr = np.searchsorted(batch, np.arange(G + 1))  # [G+1], seg g rows

    # split at segment boundaries, balancing rows
    targets = (np.arange(N_CORES + 1) * n) // N_CORES
    g_bounds = np.zeros(N_CORES + 1, dtype=np.int64)
    g_bounds[N_CORES] = G
    for c in range(1, N_CORES):
        g = int(np.argmin(np.abs(seg_ptr.astype(np.int64) - targets[c])))
        g_bounds[c] = max(g, g_bounds[c - 1])
    row_bounds = seg_ptr[g_bounds]

    rows = np.diff(row_bounds)
    segs = np.diff(g_bounds)
    GM = int(segs.max())
    assert GM <= 128, f"too many segments on one core: {GM}"
    T = int(-(-int(rows.max()) // C))
    R = T * C

    w1s = np.ascontiguousarray(W1.reshape(2, 128, H).astype(BF16))
    w2s = np.ascontiguousarray(W2.astype(BF16))
    b1s = np.ascontiguousarray(b1.reshape(H, 1))
    iota = np.broadcast_to(
        np.arange(GM, dtype=np.float32)[None, :], (128, GM)
    ).copy()

    in_maps = []
    for c in range(N_CORES):
        r0, r1 = int(row_bounds[c]), int(row_bounds[c + 1])
        g0, g1 = int(g_bounds[c]), int(g_bounds[c + 1])
        nr = r1 - r0
        xpad = np.zeros((R, D), dtype=np.float32)
        xpad[:nr] = x[r0:r1]
        # natural layout + ones column: [128, T, SUB, 257] bf16
        xnb = np.empty((R, D + 1), dtype=BF16)
        xnb[:, :D] = xpad.astype(BF16)
        xnb[:, D] = np.float32(1.0)
        xn = np.ascontiguousarray(
            xnb.reshape(T, SUB, 128, D + 1).transpose(2, 0, 1, 3)
        )
        # transposed layout: [128, T, 2, C] fp8 e3m4
        xte = xpad.astype(E3M4)
        xt = np.ascontiguousarray(
            xte.reshape(T, C, 2, 128).transpose(3, 0, 2, 1)
        )
        blo = np.empty((128, T * SUB), dtype=np.float32)
        blp = np.full(R, -1.0, dtype=np.float32)
        blp[:nr] = (batch[r0:r1] - g0).astype(np.float32)
        blo[:, :] = blp.reshape(T * SUB, 128).T
        # 1.0 for empty or padded segments (their denominator is 0)
        seg_count = np.zeros(GM, dtype=np.int64)
        cnts = seg_ptr[g0 + 1 : g1 + 1] - seg_ptr[g0:g1]
        seg_count[: g1 - g0] = cnts
        dbi = (seg_count == 0).astype(np.float32).reshape(GM, 1)
        in_maps.append(
            {
                "xn": xn,
                "xt": xt,
                "w1": w1s,
                "w2": w2s,
                "b1v": b1s,
                "blo": blo,
                "iot": iota,
                "dbi": dbi,
            }
        )

    meta = {
        "T": T,
        "GM": GM,
        "g_bounds": g_bounds,
        "G": G,
        "n": n,
    }
    return meta, in_maps


def _gather(meta, res):
    G = meta["G"]
    g_bounds = meta["g_bounds"]
    full = np.zeros((G, D), dtype=np.float32)
    for c in range(N_CORES):
        g0, g1 = int(g_bounds[c]), int(g_bounds[c + 1])
        if g1 > g0:
            full[g0:g1] = res.results[c]["out"][: g1 - g0]
    return full


def _sane(full):
    # output rows are convex combinations of x rows (|x| < ~6); a device
    # glitch shows up as a huge value or NaN.
    return bool(np.isfinite(full).all() and np.abs(full).max() < 64.0)


def _run(inputs, trace=False):
    meta, in_maps = _prepare(inputs)
    nc = _build_program(meta["T"], meta["GM"])
    try:
        res = run_bass_kernel_spmd(nc, in_maps, list(range(N_CORES)), trace=trace)
        full = _gather(meta, res)
        if not _sane(full):
            raise RuntimeError("insane output, retrying once")
    except Exception:
        # transient device failures (e.g. NRT_EXEC_UNIT_UNRECOVERABLE) happen;
        # one rebuild+retry
        nc = _build_program(meta["T"], meta["GM"])
        res = run_bass_kernel_spmd(nc, in_maps, list(range(N_CORES)), trace=trace)
        full = _gather(meta, res)
    return full, res


def kernel(**inputs) -> np.ndarray:
    out, _ = _run(inputs, trace=False)
    return out


def kernel_traced(**inputs):
    """Returns (output, BassKernelResults with exec_time_ns/profile)."""
    out, res = _run(inputs, trace=True)
    return out, res


# revision 6
# speedup vs baseline: 1.2681x; 1.2681x over previous
"""AttentionPooling (segment softmax-pool) Trainium2 Bass kernel. v3

out[g, :] = sum_{i: batch[i]==g} softmax_within_segment(score)_i * x[i, :]
score_i = tanh(x_i @ W1 + b1) @ W2 + b2

Math notes:
- softmax is shift-invariant, so b2 and the per-segment max subtraction
  cancel exactly; we compute e_i = exp(s_i) with s_i = tanh(xW1+b1)@W2
  and normalize by the per-segment sum of e at the very end.
- per-segment sums run on the TensorEngine: for each 128-node subtile,
  a one-hot matrix ow[i, g] = e_i * (batch_local[i] == g) is built with
  one DVE tensor_scalar op, then ow.T @ [x | 1] accumulates
  [num_local_segments, D+1] into ONE persistent PSUM accumulation group
  spanning the whole kernel (column D is the softmax denominator).

Precision: pooling consumes x in bf16; the score MLP consumes a second,
transposed copy of x in fp8 e3m4 (softmax weights tolerate ~2% score
noise; validated rel_err ~8e-3 vs the f32 reference). The score matmul
runs mixed-dtype: bf16 W1 stationary x fp8e3 moving.

Pipelining: the PE queue is strict FIFO, so the emission order is
software-pipelined with a 2-stage skew -- at step j we emit W1(j) and
tanh(j), score(j-1)/exp(j-1)/ow(j-1), and pool(j-2). Every PE
instruction's cross-engine inputs were produced a full step earlier,
so the PE never stalls on ScalarE/DVE.

HBM traffic per core: 12.9 MB (natural bf16 + ones col) + 6.3 MB
(transposed fp8) = 19.2 MB, moved with ~0.5-2 MB grouped DMAs (each
DMA instruction is split across all 16 SDMA engines).

Sharding: nodes split across 8 cores at segment boundaries (batch is
sorted); each core reduces its own segments; host concatenates the
per-core [G_c, D] outputs.
"""

import sys

sys.path.insert(0, "/opt/trn_rl_repo")

import numpy as np
import ml_dtypes

import concourse.bass as bass
import concourse.tile as tile
from concourse import mybir
from concourse.bass_utils import run_bass_kernel_spmd

BF16 = ml_dtypes.bfloat16
E3M4 = ml_dtypes.float8_e3m4

N_CORES = 8
D = 256
H = 128  # hidden dim of the score MLP
C = 512  # nodes per chunk
SUB = C // 128


def _split_multiwait(nc):
    """Split multi-wait instructions for this walrus build.

    This neuronxcc/walrus rejects more than one sync-wait command per
    instruction ("Too many sync wait commands"), but tile emits 2-3 waits
    on compute/DMA instructions and many on the final Drain. Hoist the
    extra waits onto preceding InstEventSemaphore instructions (the native
    sequencer wait primitive, 2 waits each) on the same engine. Engine
    program order makes this equivalent: the stream blocks on the EVSEM
    waits, then on the instruction's remaining wait.
    """
    for bb in nc.main_func.blocks:
        new = []
        for ins in bb.instructions:
            w = (
                list(ins.sync_info.on_wait)
                if (ins.sync_info and ins.sync_info.on_wait)
                else []
            )
            if len(w) > 1:
                extras = w[:-1]
                for i in range(0, len(extras), 2):
                    ev = mybir.InstEventSemaphore(
                        name=nc.get_next_instruction_name(),
                        engine=ins.engine,
                        sync_info=mybir.SyncInfo(
                            on_wait=extras[i : i + 2], on_update=[]
                        ),
                    )
                    nc.register_instruction(ev)
                    new.append(ev)
                ins.sync_info.on_wait = [w[-1]]
            new.append(ins)
        bb.instructions[:] = new


def _group_plan(T):
    """DMA grouping: small leading groups to prime the pipeline, then big."""
    plan = []
    t = 0
    for n in (1, 2, 4):
        if t >= T:
            break
        n = min(n, T - t)
        plan.append((t, t + n))
        t += n
    while t < T:
        n = min(8, T - t)
        plan.append((t, t + n))
        t += n
    return plan


def _build_program(T, GM):
    """Build the SPMD Bass program: T chunks of C nodes, GM local segments."""
    f32 = mybir.dt.float32
    bf16 = mybir.dt.bfloat16
    fp8 = mybir.dt.float8e3
    T4 = T * SUB

    nc = bass.Bass(trn_type="TRN2")
    xn = nc.dram_tensor("xn", [128, T, SUB, D + 1], bf16, kind="ExternalInput")
    xt = nc.dram_tensor("xt", [128, T, 2, C], fp8, kind="ExternalInput")
    # merged consts: bf16 [128, 2H+1] = W1 k-tiles | W2 column
    mcb = nc.dram_tensor("mcb", [128, 2 * H + 1], bf16, kind="ExternalInput")
    # merged consts: f32 [128, 1+T4+GM+1] = b1 | batch-local | iota | dbi
    mcf = nc.dram_tensor(
        "mcf", [128, 1 + T4 + GM + 1], f32, kind="ExternalInput"
    )
    out = nc.dram_tensor("out", [GM, D], f32, kind="ExternalOutput")

    Exp = mybir.ActivationFunctionType.Exp
    Tanh = mybir.ActivationFunctionType.Tanh

    plan = _group_plan(T)

    with tile.TileContext(nc) as tc:
        with (
            tc.tile_pool(name="const", bufs=1) as const,
            tc.tile_pool(name="work", bufs=8) as work,
            tc.tile_pool(name="hps", bufs=3, space="PSUM") as hps,
            tc.tile_pool(name="sps", bufs=2, space="PSUM") as sps,
            tc.tile_pool(name="accp", bufs=1, space="PSUM") as accp,
        ):
            # first x group goes first so the PE can start ASAP; then the
            # small consts; then the remaining groups.
            xtg = [
                const.tile([128, t1 - t0, 2, C], fp8, name=f"xtg{gi}")
                for gi, (t0, t1) in enumerate(plan)
            ]
            xng = [
                const.tile([128, t1 - t0, SUB, D + 1], bf16, name=f"xng{gi}")
                for gi, (t0, t1) in enumerate(plan)
            ]
            gidx = []
            for gi, (t0, t1) in enumerate(plan):
                for lt in range(t1 - t0):
                    gidx.append((gi, lt))

            nc.sync.dma_start(out=xtg[0], in_=xt[:, plan[0][0] : plan[0][1]])
            mcbs = const.tile([128, 2 * H + 1], bf16)
            nc.sync.dma_start(out=mcbs, in_=mcb[:, :])
            mcfs = const.tile([128, 1 + T4 + GM + 1], f32)
            nc.sync.dma_start(out=mcfs, in_=mcf[:, :])
            nc.sync.dma_start(out=xng[0], in_=xn[:, plan[0][0] : plan[0][1]])
            for gi, (t0, t1) in enumerate(plan):
                if gi == 0:
                    continue
                nc.sync.dma_start(out=xtg[gi], in_=xt[:, t0:t1])
                nc.sync.dma_start(out=xng[gi], in_=xn[:, t0:t1])

            w1a = mcbs[:, 0:H]
            w1b = mcbs[:, H : 2 * H]
            w2sb = mcbs[:, 2 * H : 2 * H + 1]
            b1sb = mcfs[:, 0:1]
            blsb = mcfs[:, 1 : 1 + T4]
            iosb = mcfs[:, 1 + T4 : 1 + T4 + GM]
            dbsb = mcfs[:, 1 + T4 + GM : 1 + T4 + GM + 1]

            # persistent PSUM accumulator: one accumulation group spans
            # every pooling matmul in the kernel.
            pch = accp.tile([GM, D + 1], f32)

            hp_t = [None] * T
            tht_t = [None] * T
            sp_t = [None] * T
            et_t = [None] * T
            owt_t = [None] * T

            for j in range(T + 2):
                # stage A: W1 matmul + tanh for chunk j
                if j < T:
                    gi, lt = gidx[j]
                    hp = hps.tile([H, C], f32)
                    nc.tensor.matmul(
                        hp,
                        lhsT=w1a,
                        rhs=xtg[gi][:, lt, 0, :],
                        start=True,
                        stop=False,
                    )
                    nc.tensor.matmul(
                        hp,
                        lhsT=w1b,
                        rhs=xtg[gi][:, lt, 1, :],
                        start=False,
                        stop=True,
                    )
                    tht = work.tile([H, C], bf16)
                    nc.scalar.activation(tht, hp, Tanh, bias=b1sb)
                    hp_t[j], tht_t[j] = hp, tht

                # stage B: scores + exp + one-hot weights for chunk j-1
                if 0 <= j - 1 < T:
                    jb = j - 1
                    tht = tht_t[jb]
                    sp = sps.tile([128, SUB], f32)
                    nc.vector.memset(sp, 0.0)
                    for a in range(SUB):
                        nc.tensor.matmul(
                            sp[:, a : a + 1],
                            lhsT=tht[:, a * 128 : (a + 1) * 128],
                            rhs=w2sb,
                            start=False,
                            stop=(a == SUB - 1),
                            skip_group_check=True,
                        )
                    et = work.tile([128, SUB], f32)
                    nc.scalar.activation(et, sp, Exp)
                    owt = work.tile([128, SUB, GM], bf16)
                    for a in range(SUB):
                        t4 = jb * SUB + a
                        nc.vector.tensor_scalar(
                            out=owt[:, a, :],
                            in0=iosb,
                            scalar1=blsb[:, t4 : t4 + 1],
                            scalar2=et[:, a : a + 1],
                            op0=mybir.AluOpType.is_equal,
                            op1=mybir.AluOpType.mult,
                        )
                    sp_t[jb], et_t[jb], owt_t[jb] = sp, et, owt

                # stage C: pooling matmuls for chunk j-2 (ow ready a full
                # step ago, so the PE never waits here)
                if 0 <= j - 2:
                    jc = j - 2
                    gi, lt = gidx[jc]
                    owt = owt_t[jc]
                    for a in range(SUB):
                        nc.tensor.matmul(
                            pch,
                            lhsT=owt[:, a, :],
                            rhs=xng[gi][:, lt, a, :],
                            start=(jc == 0 and a == 0),
                            stop=(jc == T - 1 and a == SUB - 1),
                            skip_group_check=True,
                        )

            # normalize: out[g, :] = pooled[g, :D] / (denom[g] + empty_guard)
            dn = work.tile([GM, 1], f32)
            nc.vector.tensor_scalar_add(dn, pch[:, D : D + 1], dbsb[0:GM, 0:1])
            rc = work.tile([GM, 1], f32)
            nc.vector.reciprocal(rc, dn)
            ot = work.tile([GM, D], f32)
            nc.vector.tensor_scalar_mul(ot, pch[:, 0:D], rc[:, 0:1])
            nc.sync.dma_start(out=out[:, :], in_=ot)

    _split_multiwait(nc)
    return nc


def _prepare(inputs):
    """Host-side sharding and input staging. Returns (meta, in_maps)."""
    x = np.asarray(inputs["x"], dtype=np.float32)
    batch = np.asarray(inputs["batch"]).astype(np.int64)
    W1 = np.asarray(inputs["W1"], dtype=np.float32)
    b1 = np.asarray(inputs["b1"], dtype=np.float32)
    W2 = np.asarray(inputs["W2"], dtype=np.float32)

    n, d = x.shape
    assert d == D
    G = 512
    seg_ptr = np.searchsorted(batch, np.arange(G + 1))  # [G+1], seg g rows

    # split at segment boundaries, balancing rows
    targets = (np.arange(N_CORES + 1) * n) // N_CORES
    g_bounds = np.zeros(N_CORES + 1, dtype=np.int64)
    g_bounds[N_CORES] = G
    for c in range(1, N_CORES):
        g = int(np.argmin(np.abs(seg_ptr.astype(np.int64) - targets[c])))
        g_bounds[c] = max(g, g_bounds[c - 1])
    row_bounds = seg_ptr[g_bounds]

    rows = np.diff(row_bounds)
    segs = np.diff(g_bounds)
    GM = int(segs.max())
    assert GM <= 128, f"too many segments on one core: {GM}"
    T = int(-(-int(rows.max()) // C))
    R = T * C
    T4 = T * SUB

    # merged bf16 consts: [128, 2H+1] = W1 halves | W2
    mcb = np.zeros((128, 2 * H + 1), dtype=BF16)
    mcb[:, 0:H] = W1[0:128].astype(BF16)
    mcb[:, H : 2 * H] = W1[128:256].astype(BF16)
    mcb[:, 2 * H] = W2[:, 0].astype(BF16)

    iota = np.arange(GM, dtype=np.float32)

    in_maps = []
    for c in range(N_CORES):
        r0, r1 = int(row_bounds[c]), int(row_bounds[c + 1])
        g0, g1 = int(g_bounds[c]), int(g_bounds[c + 1])
        nr = r1 - r0
        xpad = np.zeros((R, D), dtype=np.float32)
        xpad[:nr] = x[r0:r1]
        # natural layout + ones column: [128, T, SUB, 257] bf16
        xnb = np.empty((R, D + 1), dtype=BF16)
        xnb[:, :D] = xpad.astype(BF16)
        xnb[:, D] = np.float32(1.0)
        xnh = np.ascontiguousarray(
            xnb.reshape(T, SUB, 128, D + 1).transpose(2, 0, 1, 3)
        )
        # transposed layout: [128, T, 2, C] fp8 e3m4
        xte = xpad.astype(E3M4)
        xth = np.ascontiguousarray(
            xte.reshape(T, C, 2, 128).transpose(3, 0, 2, 1)
        )
        blp = np.full(R, -1.0, dtype=np.float32)
        blp[:nr] = (batch[r0:r1] - g0).astype(np.float32)
        # 1.0 for empty or padded segments (their denominator is 0)
        seg_count = np.zeros(GM, dtype=np.int64)
        cnts = seg_ptr[g0 + 1 : g1 + 1] - seg_ptr[g0:g1]
        seg_count[: g1 - g0] = cnts
        # merged f32 consts: [128, 1+T4+GM+1] = b1 | bl | iota | dbi
        mcf = np.zeros((128, 1 + T4 + GM + 1), dtype=np.float32)
        mcf[:, 0] = b1
        mcf[:, 1 : 1 + T4] = blp.reshape(T4, 128).T
        mcf[:, 1 + T4 : 1 + T4 + GM] = iota[None, :]
        mcf[:GM, 1 + T4 + GM] = (seg_count == 0).astype(np.float32)
        in_maps.append(
            {"xn": xnh, "xt": xth, "mcb": mcb, "mcf": mcf}
        )

    meta = {
        "T": T,
        "GM": GM,
        "g_bounds": g_bounds,
        "G": G,
        "n": n,
    }
    return meta, in_maps


def _gather(meta, res):
    G = meta["G"]
    g_bounds = meta["g_bounds"]
    full = np.zeros((G, D), dtype=np.float32)
    for c in range(N_CORES):
        g0, g1 = int(g_bounds[c]), int(g_bounds[c + 1])
        if g1 > g0:
            full[g0:g1] = res.results[c]["out"][: g1 - g0]
    return full


def _sane(full):
    # output rows are convex combinations of x rows (|x| < ~6); a device
    # glitch shows up as a huge value or NaN.
    return bool(np.isfinite(full).all() and np.abs(full).max() < 64.0)


def _run(inputs, trace=False):
    meta, in_maps = _prepare(inputs)
    nc = _build_program(meta["T"], meta["GM"])
    try:
        res = run_bass_kernel_spmd(nc, in_maps, list(range(N_CORES)), trace=trace)
        full = _gather(meta, res)
        if not _sane(full):
            raise RuntimeError("insane output, retrying once")
    except Exception:
        # transient device failures (e.g. NRT_EXEC_UNIT_UNRECOVERABLE) happen;
        # one rebuild+retry
        nc = _build_program(meta["T"], meta["GM"])
        res = run_bass_kernel_spmd(nc, in_maps, list(range(N_CORES)), trace=trace)
        full = _gather(meta, res)
    return full, res


def kernel(**inputs) -> np.ndarray:
    out, _ = _run(inputs, trace=False)
    return out


def kernel_traced(**inputs):
    """Returns (output, BassKernelResults with exec_time_ns/profile)."""
    out, res = _run(inputs, trace=True)
    return out, res
